# revision 23
# baseline (speedup 1.0000x reference)
"""Trainium2 Bass kernel for nn_Attention_18176301596931.

Dense GQA attention layer (B=1, S=2048, D=2048, 32 Q heads / 8 KV heads,
HD=64, interleaved RoPE, causal softmax) tensor-parallel over 8 NeuronCores:
core i owns Q heads 4i..4i+3 and KV head i. Each core computes its heads'
QKV projection, RoPE, attention, and a partial output projection over its
256 columns of wo; the host sums the 8 fp16 partial outputs in fp32.

v3 (vs v2): rebalances work off the Vector engine (56% busy in the v2
trace) and keeps the PE stream dense (v2 lost ~43us to HAM half-clock).
 - q/k stored fp16 (was float32r): FWL weight loads on the scores matmuls,
   full-rate at any moving size (diagonal blocks shrink to their live
   range), half the SBUF.
 - RoPE pairs de-interleaved at 16-lane granularity so the real/imag swap
   is a single DVE stream_shuffle (lane XOR 16 within each 32-partition
   group) instead of four 32-row cross-partition ops; the evict to fp16
   runs on ACT, the two table muls on DVE at 16-bit rate, the final add
   on GpSimd (otherwise idle).
 - causal mask shrunk to the 128-wide diagonal triangle, both halves
   masked in one 3D-AP op against a single [128,2,128] table.
 - diagonal-block exps batched across the two heads with a 3D AP.
 - softmax normalization: reciprocal reads the PSUM denominator row
   directly, GpSimd partition_broadcast replaces the ones-column PE
   matmul + PSUM->SBUF copy, leaving one DVE mul per half.
 - output-projection PSUM evictions alternate ACT/DVE instead of all-DVE.
 - output DMAs moved to the sync queue (GpSimd runs rope adds/broadcasts).

Matmul dtypes: fp16 throughout (inputs quantized host-side); scores/attn
accumulate fp32 in PSUM. Softmax skips max-subtraction (logits ~ N(0,1);
exp stays in range in fp16). The denominator rides the attn@V matmul as a
65th output row via a ones column appended to V.
"""

import numpy as np
import jax
from jax.sharding import Mesh, PartitionSpec
from jax.experimental.shard_map import shard_map

import concourse.bass as bass
import concourse.mybir as mybir
import concourse.tile as tile
from concourse.bacc import Bacc
from concourse import bass2jax
from concourse.bass2jax import (
    _bass_exec_p,
    install_neuronx_cc_hook,
    partition_id_tensor,
)

F32 = mybir.dt.float32
F16 = mybir.dt.float16

B, S, D = 1, 2048, 2048
NH, NKV, HD = 32, 8, 64
N_CORES = 8
HQ = NH // N_CORES          # 4 q heads per core
EQ = HQ * HD                # 256 q columns per core
ECORE = EQ + 2 * HD         # 384 qkv columns per core
DO = D // 128               # 16 contraction chunks
SW = 512                    # seq window (matmul moving dim)
NG = S // SW                # 4 groups
NSB = S // 128              # 16 sk blocks
SCALE = 1.0 / np.sqrt(HD)
SWAP16 = list(range(16, 32)) + list(range(16))


def _build(loop: int = 1) -> bass.Bass:
    nc = Bacc()
    x_ext = nc.declare_dram_parameter("xt", [128, NG * DO * SW], F16, isOutput=False)
    w_ext = nc.declare_dram_parameter("wt", [128, DO * ECORE], F16, isOutput=False)
    wo_ext = nc.declare_dram_parameter("wot", [128, 2, D], F16, isOutput=False)
    a_ext = nc.declare_dram_parameter("ropea", [128, S], F16, isOutput=False)
    b_ext = nc.declare_dram_parameter("ropeb", [128, S], F16, isOutput=False)
    t_ext = nc.declare_dram_parameter("trim", [128, 2, 128], F16, isOutput=False)
    id_ext = nc.declare_dram_parameter("ident", [HD, HD], F16, isOutput=False)
    vo_ext = nc.declare_dram_parameter("vones", [128, NSB, 1], F16, isOutput=False)
    out_ext = nc.declare_dram_parameter("out", [S, D], F16, isOutput=True)

    with (
        nc.allow_low_precision(reason="fp16 storage is intentional"),
        tile.TileContext(nc) as tc,
        tc.tile_pool(name="const", bufs=1) as constp,
        tc.tile_pool(name="persist", bufs=1) as persist,
        tc.tile_pool(name="xs", bufs=4) as xsp,
        tc.tile_pool(name="rope", bufs=4) as ropep,
        tc.tile_pool(name="work", bufs=4) as work,
        tc.tile_pool(name="ot", bufs=2) as otp,
        tc.tile_pool(name="pt", bufs=6) as ptp,
        tc.tile_pool(name="proj", bufs=2, space="PSUM") as projp,
        tc.tile_pool(name="pssc", bufs=2, space="PSUM") as pssc,
        tc.tile_pool(name="psy", bufs=2, space="PSUM") as psyp,
    ):
        ident = constp.tile([HD, HD], F16)
        ropea = constp.tile([128, S], F16)
        ropeb = constp.tile([128, S], F16)
        tri = constp.tile([128, 2, 128], F16)
        wot = constp.tile([128, 2, D], F16)
        wt_sb = constp.tile([128, DO * ECORE], F16)
        # spread const loads across engine DMA queues, first-needed first
        WQ = DO * ECORE // 2
        for q in range(2):
            nc.scalar.dma_start(
                wt_sb[:, WQ * q : WQ * (q + 1)], w_ext[:, WQ * q : WQ * (q + 1)]
            )
        nc.gpsimd.dma_start(ropea[:], a_ext[:])
        nc.gpsimd.dma_start(ropeb[:], b_ext[:])
        nc.gpsimd.dma_start(ident[:], id_ext[:])
        nc.gpsimd.dma_start(tri[:, :, :], t_ext[:, :, :])
        nc.gpsimd.dma_start(wot[:], wo_ext[:])

        def warmup():
            # keep the PE HAM activity window busy during the input-DMA wait
            # so the first real chains run at the full 2.4 GHz clock
            wrm = work.tile([128, 128], F16, name="wrm")
            nc.vector.memset(wrm[:], 0.0)
            pw = projp.tile([128, SW], F32, name="pj")
            for r in range(40):
                nc.tensor.matmul(
                    pw[:, :128], wrm[:], wrm[:], start=(r == 0), stop=(r == 39)
                )

        def body(_i=None):
            # qq[t]: packed RoPE'd q head-pair tiles [128, S] (heads 2t, 2t+1,
            # 16-granular de-interleave); kT2: RoPE'd k duplicated in both
            # partition halves so matmul base-alignment works for odd heads.
            qq = [persist.tile([128, S], F16, name=f"qq{t}") for t in range(2)]
            kT2 = persist.tile([128, S], F16, name="kT2")
            vTh = persist.tile([HD, S], F16, name="vTh")
            v_sk = persist.tile([128, NSB, HD + 1], F16, name="v_sk")
            nc.gpsimd.dma_start(v_sk[:, :, HD : HD + 1], vo_ext[:])
            yT = [persist.tile([128, S], F16, name=f"yT{c}") for c in range(2)]

            def rope_evict(dst, ps, rows, cols, g):
                # dst = ps*A + lane_swap16(ps)*B  (rows 0:64 or 0:128)
                p16 = ropep.tile([128, SW], F16, name="p16")
                nc.scalar.copy(p16[:rows, :], ps[:rows, :])
                sw = ropep.tile([128, SW], F16, name="sw")
                nc.vector.stream_shuffle(sw[:rows, :], p16[:rows, :], SWAP16)
                u = ropep.tile([128, SW], F16, name="u")
                nc.vector.tensor_mul(
                    out=u[:rows, :], in0=p16[:rows, :], in1=ropea[:rows, cols]
                )
                v = ropep.tile([128, SW], F16, name="v")
                nc.vector.tensor_mul(
                    out=v[:rows, :], in0=sw[:rows, :], in1=ropeb[:rows, cols]
                )
                nc.gpsimd.tensor_add(out=dst, in0=u[:rows, :], in1=v[:rows, :])

            def qkv_group(g):
                cols = slice(g * SW, (g + 1) * SW)
                xs = xsp.tile([128, DO * SW], F16, name="xs")
                XQ = DO * SW // 2
                eng = nc.scalar if g == 1 else nc.sync
                for q in range(2):
                    eng.dma_start(
                        xs[:, XQ * q : XQ * (q + 1)],
                        x_ext[:, g * DO * SW + XQ * q : g * DO * SW + XQ * (q + 1)],
                    )
                for e in (2, 0, 1):  # k/v first: attention needs kT2 earliest
                    ps = projp.tile([128, SW], F32, name="pj")
                    for ko in range(DO):
                        nc.tensor.matmul(
                            ps,
                            wt_sb[:, ko * ECORE + e * 128 : ko * ECORE + (e + 1) * 128],
                            xs[:, ko * SW : (ko + 1) * SW],
                            start=(ko == 0),
                            stop=(ko == DO - 1),
                        )
                    if e < 2:
                        rope_evict(qq[e][:, cols], ps, 128, cols, g)
                    else:
                        rope_evict(kT2[0:HD, cols], ps, HD, cols, g)
                        nc.scalar.copy(kT2[HD:128, cols], kT2[0:HD, cols])
                        nc.vector.tensor_copy(vTh[:, cols], ps[HD:128, :])
                # ---- V transpose for this group's sk chunks ----
                for c in range(4 * g, 4 * g + 4):
                    pv = projp.tile([128, SW], F32, name="pj").bitcast(F16)[:, :HD]
                    nc.tensor.transpose(
                        pv[:], vTh[:, c * 128 : (c + 1) * 128], ident[:]
                    )
                    nc.vector.tensor_copy(v_sk[:, c, 0:HD], pv[:])

            def attn_pair(g, t):
                nblk = 4 * (g + 1)
                gwin = slice(g * SW, (g + 1) * SW)
                psy2 = [psyp.tile([HD + 1, SW], F32, name="y") for _ in range(2)]
                for b in range(nblk):
                    j = b - (nblk - 4)
                    blk = slice(b * 128, (b + 1) * 128)
                    # live subrange of the 512-wide sq window for this block
                    lsub = slice(128 * j, SW) if j > 0 else slice(0, SW)
                    pss = pssc.tile([128, 2, SW], F32, name="sc")
                    for half in range(2):
                        r = half * HD
                        nc.tensor.matmul(
                            pss[:, half, lsub],
                            kT2[r : r + HD, blk],
                            qq[t][r : r + HD, gwin][:, lsub],
                            start=True,
                            stop=True,
                            tile_position=(r, 0),
                        )
                    pt = ptp.tile([128, 2, SW], F16, name="pt")
                    if j >= 1:
                        nc.scalar.activation(
                            pt[:, :, lsub],
                            pss[:, :, lsub],
                            mybir.ActivationFunctionType.Exp,
                            scale=float(SCALE),
                        )
                    else:
                        nc.scalar.activation(
                            pt[:, :, :],
                            pss[:, :, :],
                            mybir.ActivationFunctionType.Exp,
                            scale=float(SCALE),
                        )
                    if j >= 0:
                        dsub = slice(128 * j, 128 * (j + 1))
                        nc.vector.tensor_mul(
                            out=pt[:, :, dsub], in0=pt[:, :, dsub], in1=tri[:, :, :]
                        )
                    for half in range(2):
                        nc.tensor.matmul(
                            psy2[half][:, lsub],
                            v_sk[:, b, :],
                            pt[:, half, lsub],
                            start=(b == 0),
                            stop=(b == nblk - 1),
                        )
                # ---- normalization: yT = psy / denom ----
                # evict psy to SBUF first so the PSUM banks free immediately
                # (the next pair's attn@V is gated on them); the reciprocal/
                # broadcast/divide then run off the critical path.
                for half in range(2):
                    psy = psy2[half]
                    base = half * HD
                    ev = work.tile([HD, SW], F32, name="ev")
                    nc.vector.tensor_copy(ev[:, :], psy[0:HD, :])
                    # denominator row to partition 0: reciprocal_approx_fast
                    # misbehaves on HW with a nonzero input partition base
                    dnr = work.tile([1, SW], F32, name="dnr")
                    nc.vector.tensor_copy(dnr[:], psy[HD : HD + 1, :])
                    rec = work.tile([1, SW], F32, name="rec")
                    nc.vector.reciprocal_approx_fast(rec[:], dnr[:])
                    bcs = work.tile([HD, SW], F32, name="bcs")
                    nc.gpsimd.partition_broadcast(bcs[:, :], rec[:, :], channels=HD)
                    nc.vector.tensor_mul(
                        out=yT[t][base : base + HD, gwin],
                        in0=ev[:, :],
                        in1=bcs[:, :],
                    )

            def wo_group(g):
                for sq in range(4 * g, 4 * g + 4):
                    ot = otp.tile([128, D], F16, name="ot")
                    for do in range(NG):
                        pso = projp.tile([128, SW], F32, name="pj")
                        for c in range(2):
                            nc.tensor.matmul(
                                pso[:],
                                yT[c][:, sq * 128 : (sq + 1) * 128],
                                wot[:, c, do * SW : (do + 1) * SW],
                                start=(c == 0),
                                stop=(c == 1),
                            )
                        if do == 0:
                            nc.scalar.copy(ot[:, do * SW : (do + 1) * SW], pso[:])
                        else:
                            nc.vector.tensor_copy(
                                ot[:, do * SW : (do + 1) * SW], pso[:]
                            )
                    nc.gpsimd.dma_start(
                        out_ext[sq * 128 : (sq + 1) * 128, :], ot[:]
                    )

            # interleave: PE filler work (qkv projections / output proj) is
            # emitted BETWEEN the two attention pairs of each group so the
            # scheduler can cover each pair's serial softmax tail.
            qkv_group(0)
            qkv_group(1)
            for g in range(NG):
                attn_pair(g, 0)
                if g < NG - 2:
                    qkv_group(g + 2)
                elif g >= 1:
                    wo_group(g - 1)
                attn_pair(g, 1)
                if g == 1:
                    wo_group(0)
            wo_group(NG - 1)

        warmup()
        if loop <= 1:
            body()
        else:
            with tc.For_i(0, loop, 1) as i:
                body(i)
    nc.finalize()
    return nc


class _CompiledSpmd:
    def __init__(self, nc: bass.Bass, n_cores: int = N_CORES):
        install_neuronx_cc_hook()
        self.nc = nc
        self.n_cores = n_cores
        partition_name = nc.partition_id_tensor.name if nc.partition_id_tensor else None

        in_names, out_names, out_avals, zero_shapes = [], [], [], []
        for alloc in nc.m.functions[0].allocations:
            if not isinstance(alloc, mybir.MemoryLocationSet):
                continue
            name = alloc.memorylocations[0].name
            if alloc.kind == "ExternalInput":
                if name != partition_name and name != (
                    nc.dbg_addr.name if nc.dbg_addr else None
                ):
                    in_names.append(name)
            elif alloc.kind == "ExternalOutput":
                out_names.append(name)
                shape = tuple(alloc.tensor_shape)
                dtype = mybir.dt.np(alloc.dtype)
                out_avals.append(jax.core.ShapedArray(shape, dtype))
                zero_shapes.append((shape, dtype))

        self.in_names, self.out_names = in_names, out_names
        self.out_avals, self.zero_shapes = out_avals, zero_shapes
        n_params, n_outs = len(in_names), len(out_names)

        full_in_names = list(in_names) + list(out_names)
        if nc.dbg_addr is not None:
            full_in_names.append(nc.dbg_addr.name)
        if partition_name is not None:
            full_in_names.append(partition_name)
        has_dbg = nc.dbg_addr is not None

        def _body(*args):
            operands = list(args)
            if has_dbg:
                operands.append(np.zeros((1, 2), np.uint32))
            if partition_name is not None:
                operands.append(partition_id_tensor())
            return tuple(
                _bass_exec_p.bind(
                    *operands,
                    out_avals=tuple(out_avals),
                    in_names=tuple(full_in_names),
                    out_names=tuple(out_names),
                    lowering_input_output_aliases=(),
                    sim_require_finite=True,
                    sim_require_nnan=True,
                    nc=nc,
                )
            )

        donate = tuple(range(n_params, n_params + n_outs))
        devices = jax.devices()[:n_cores]
        mesh = Mesh(np.asarray(devices), ("core",))
        self._fn = jax.jit(
            shard_map(
                _body,
                mesh=mesh,
                in_specs=(PartitionSpec("core"),) * (n_params + n_outs),
                out_specs=(PartitionSpec("core"),) * n_outs,
                check_rep=False,
            ),
            donate_argnums=donate,
            keep_unused=True,
        )

    def prep_inputs(self, in_maps):
        n = self.n_cores
        concat = [
            np.concatenate([np.asarray(in_maps[c][name]) for c in range(n)], axis=0)
            for name in self.in_names
        ]
        return [jax.device_put(a) for a in concat]

    def _zeros(self):
        return [
            np.zeros((self.n_cores * s[0], *s[1:]), d) for s, d in self.zero_shapes
        ]

    def run_prepped(self, dev_inputs):
        out = self._fn(*dev_inputs, *self._zeros())
        jax.block_until_ready(out)
        return out

    def run(self, in_maps):
        out_arrs = self.run_prepped(self.prep_inputs(in_maps))
        n = self.n_cores
        return [
            {
                name: np.asarray(out_arrs[i]).reshape(n, *self.out_avals[i].shape)[c]
                for i, name in enumerate(self.out_names)
            }
            for c in range(n)
        ]


# de-interleave pairs at 16-lane granularity: per 32-partition group,
# lanes 0:16 = real parts, 16:32 = imag parts (swap partner = lane XOR 16)
_PERM = np.concatenate(
    [
        np.arange(0, 32, 2),      # real of pairs 0..15
        np.arange(1, 32, 2),      # imag of pairs 0..15
        np.arange(32, 64, 2),     # real of pairs 16..31
        np.arange(33, 64, 2),     # imag of pairs 16..31
    ]
)


def make_in_maps(x, cos, sin, wqkv, wo):
    x = np.asarray(x, np.float32)
    cos = np.asarray(cos, np.float32)
    sin = np.asarray(sin, np.float32)
    wqkv = np.asarray(wqkv, np.float32)
    wo = np.asarray(wo, np.float32)

    # [128, NG, DO, SW] flattened: per-group loads are fully contiguous
    xt = np.ascontiguousarray(
        x[0].T.reshape(DO, 128, NG, SW).transpose(1, 2, 0, 3).reshape(128, -1)
    ).astype(np.float16)

    cosT, sinT = cos.T, sin.T  # [32, S]
    # 32-row periodic tables matching the 16-granular de-interleave:
    # rows 32a+b: b<16 -> freqs 16a+b (cos / -sin), b>=16 -> freqs 16a+b-16
    # (cos / +sin)
    a0 = np.concatenate([cosT[0:16], cosT[0:16]], axis=0)     # [32, S]
    a1 = np.concatenate([cosT[16:32], cosT[16:32]], axis=0)
    b0 = np.concatenate([-sinT[0:16], sinT[0:16]], axis=0)
    b1 = np.concatenate([-sinT[16:32], sinT[16:32]], axis=0)
    ropea = np.ascontiguousarray(np.tile(np.vstack([a0, a1]), (2, 1))).astype(
        np.float16
    )
    ropeb = np.ascontiguousarray(np.tile(np.vstack([b0, b1]), (2, 1))).astype(
        np.float16
    )

    pp, ff = np.arange(128)[:, None], np.arange(128)[None, :]
    trione = (ff >= pp).astype(np.float16)          # [128, 128] causal triangle
    trim = np.ascontiguousarray(
        np.stack([trione, trione], axis=1)
    ).astype(np.float16)                            # [128, 2, 128] both halves

    ident = np.eye(HD, dtype=np.float16)
    vones = np.ones((128, NSB, 1), np.float16)

    in_maps = []
    for i in range(N_CORES):
        wq = wqkv[i * EQ : (i + 1) * EQ].reshape(HQ, HD, D)[:, _PERM, :].reshape(
            EQ, D
        )
        wk = wqkv[NH * HD + i * HD : NH * HD + (i + 1) * HD][_PERM]
        wv = wqkv[NH * HD + NKV * HD + i * HD : NH * HD + NKV * HD + (i + 1) * HD]
        wcore = np.concatenate([wq, wk, wv], axis=0)  # [384, D]
        wt = np.ascontiguousarray(
            wcore.T.reshape(DO, 128, ECORE).transpose(1, 0, 2).reshape(128, -1)
        ).astype(np.float16)
        wos = wo[:, i * EQ : (i + 1) * EQ]  # [D, 256]
        wot = np.ascontiguousarray(
            wos.T.reshape(2, 128, D).transpose(1, 0, 2)
        ).astype(np.float16)
        in_maps.append(
            {
                "xt": xt,
                "wt": wt,
                "wot": wot,
                "ropea": ropea,
                "ropeb": ropeb,
                "trim": trim,
                "ident": ident,
                "vones": vones,
            }
        )
    return in_maps


_CACHE = {}


def get_compiled(loop: int = 1) -> _CompiledSpmd:
    if loop not in _CACHE:
        _CACHE[loop] = _CompiledSpmd(_build(loop))
    return _CACHE[loop]


def kernel(x, cos, sin, wqkv, wo):
    comp = get_compiled(1)
    in_maps = make_in_maps(x, cos, sin, wqkv, wo)
    results = comp.run(in_maps)
    acc = results[0]["out"].astype(np.float32)
    for c in range(1, N_CORES):
        acc += results[c]["out"].astype(np.float32)
    return acc.astype(np.float32).reshape(B, S, D)


# revision 24
# speedup vs baseline: 2.1444x; 2.1444x over previous
"""Trainium2 Bass kernel for nn_Attention_18176301596931.

Dense GQA attention layer (B=1, S=2048, D=2048, 32 Q heads / 8 KV heads,
HD=64, interleaved RoPE, causal softmax) tensor-parallel over 8 NeuronCores:
core i owns Q heads 4i..4i+3 and KV head i. Each core computes its heads'
QKV projection, RoPE, attention, and a partial output projection over its
256 columns of wo; the host sums the 8 fp16 partial outputs in fp32.

v3 (vs the v2 baseline, 236.9us): rebalances work off the Vector engine
(56% busy in the v2 trace) and keeps the PE stream dense (v2 lost ~43us
to HAM half-clock).
 - q/k stored fp16 (was float32r): FWL weight loads on the scores matmuls,
   full-rate at any moving size (diagonal blocks shrink to their live
   range), half the SBUF.
 - RoPE pairs de-interleaved at 16-lane granularity so the real/imag swap
   is a single DVE stream_shuffle (lane XOR 16 within each 32-partition
   group) instead of four 32-row cross-partition ops; the evict to fp16
   runs on ACT, the two table muls on DVE at 16-bit rate, the final add
   on GpSimd (otherwise idle).
 - causal mask shrunk to the 128-wide diagonal triangle, both halves
   masked in one 3D-AP op against a single [128,2,128] table.
 - diagonal-block exps batched across the two heads with a 3D AP.
 - softmax normalization: psy evicted to SBUF immediately after the attn@V
   stop (frees the PSUM banks that gate the next pair), then reciprocal +
   GpSimd partition_broadcast + one DVE mul, replacing the ones-column PE
   broadcast matmul. The reciprocal input must sit at partition 0:
   reciprocal_approx_fast returns garbage on HW for a nonzero input
   partition base (CoreSim does not model this).
 - output-projection PSUM evictions split ACT/DVE instead of all-DVE.
 - emission interleave: qkv(g+2)/wo(g-1) emitted BETWEEN the two attention
   pairs of group g so the scheduler covers each pair's softmax tail.
   Output DMAs stay on the GpSimd queue: putting them on the sync queue
   head-of-line blocks the xs input prefetches behind ot-ready waits.

Matmul dtypes: fp16 throughout (inputs quantized host-side); scores/attn
accumulate fp32 in PSUM. Softmax skips max-subtraction (logits ~ N(0,1);
exp stays in range in fp16: max logit ~5.5 -> e^5.5 ~ 245). The
denominator rides the attn@V matmul as a 65th output row via a ones
column appended to V.
"""

import numpy as np
import jax
from jax.sharding import Mesh, PartitionSpec
from jax.experimental.shard_map import shard_map

import concourse.bass as bass
import concourse.mybir as mybir
import concourse.tile as tile
from concourse.bacc import Bacc
from concourse import bass2jax
from concourse.bass2jax import (
    _bass_exec_p,
    install_neuronx_cc_hook,
    partition_id_tensor,
)

F32 = mybir.dt.float32
F16 = mybir.dt.float16

B, S, D = 1, 2048, 2048
NH, NKV, HD = 32, 8, 64
N_CORES = 8
HQ = NH // N_CORES          # 4 q heads per core
EQ = HQ * HD                # 256 q columns per core
ECORE = EQ + 2 * HD         # 384 qkv columns per core
DO = D // 128               # 16 contraction chunks
SW = 512                    # seq window (matmul moving dim)
NG = S // SW                # 4 groups
NSB = S // 128              # 16 sk blocks
SCALE = 1.0 / np.sqrt(HD)
SWAP16 = list(range(16, 32)) + list(range(16))


def _build(loop: int = 1) -> bass.Bass:
    nc = Bacc()
    x_ext = nc.declare_dram_parameter("xt", [128, NG * DO * SW], F16, isOutput=False)
    w_ext = nc.declare_dram_parameter("wt", [128, DO * ECORE], F16, isOutput=False)
    wo_ext = nc.declare_dram_parameter("wot", [128, 2, D], F16, isOutput=False)
    a_ext = nc.declare_dram_parameter("ropea", [128, S], F16, isOutput=False)
    b_ext = nc.declare_dram_parameter("ropeb", [128, S], F16, isOutput=False)
    t_ext = nc.declare_dram_parameter("trim", [128, 2, 128], F16, isOutput=False)
    id_ext = nc.declare_dram_parameter("ident", [HD, HD], F16, isOutput=False)
    vo_ext = nc.declare_dram_parameter("vones", [128, NSB, 1], F16, isOutput=False)
    out_ext = nc.declare_dram_parameter("out", [S, D], F16, isOutput=True)

    with (
        nc.allow_low_precision(reason="fp16 storage is intentional"),
        tile.TileContext(nc) as tc,
        tc.tile_pool(name="const", bufs=1) as constp,
        tc.tile_pool(name="persist", bufs=1) as persist,
        tc.tile_pool(name="xs", bufs=4) as xsp,
        tc.tile_pool(name="rope", bufs=4) as ropep,
        tc.tile_pool(name="work", bufs=4) as work,
        tc.tile_pool(name="ot", bufs=2) as otp,
        tc.tile_pool(name="pt", bufs=6) as ptp,
        tc.tile_pool(name="proj", bufs=2, space="PSUM") as projp,
        tc.tile_pool(name="pssc", bufs=2, space="PSUM") as pssc,
        tc.tile_pool(name="psy", bufs=2, space="PSUM") as psyp,
    ):
        ident = constp.tile([HD, HD], F16)
        ropea = constp.tile([128, S], F16)
        ropeb = constp.tile([128, S], F16)
        tri = constp.tile([128, 2, 128], F16)
        wot = constp.tile([128, 2, D], F16)
        wt_sb = constp.tile([128, DO * ECORE], F16)
        # spread const loads across engine DMA queues, first-needed first
        WQ = DO * ECORE // 2
        for q in range(2):
            nc.scalar.dma_start(
                wt_sb[:, WQ * q : WQ * (q + 1)], w_ext[:, WQ * q : WQ * (q + 1)]
            )
        nc.gpsimd.dma_start(ropea[:], a_ext[:])
        nc.gpsimd.dma_start(ropeb[:], b_ext[:])
        nc.gpsimd.dma_start(ident[:], id_ext[:])
        nc.gpsimd.dma_start(tri[:, :, :], t_ext[:, :, :])
        nc.gpsimd.dma_start(wot[:], wo_ext[:])

        def warmup():
            # keep the PE HAM activity window busy during the input-DMA wait
            # so the first real chains run at the full 2.4 GHz clock
            wrm = work.tile([128, 128], F16, name="wrm")
            nc.vector.memset(wrm[:], 0.0)
            pw = projp.tile([128, SW], F32, name="pj")
            for r in range(40):
                nc.tensor.matmul(
                    pw[:, :128], wrm[:], wrm[:], start=(r == 0), stop=(r == 39)
                )

        def body(_i=None):
            # qq[t]: packed RoPE'd q head-pair tiles [128, S] (heads 2t, 2t+1,
            # 16-granular de-interleave); kT2: RoPE'd k duplicated in both
            # partition halves so matmul base-alignment works for odd heads.
            qq = [persist.tile([128, S], F16, name=f"qq{t}") for t in range(2)]
            kT2 = persist.tile([128, S], F16, name="kT2")
            vTh = persist.tile([HD, S], F16, name="vTh")
            v_sk = persist.tile([128, NSB, HD + 1], F16, name="v_sk")
            nc.gpsimd.dma_start(v_sk[:, :, HD : HD + 1], vo_ext[:])
            yT = [persist.tile([128, S], F16, name=f"yT{c}") for c in range(2)]

            def rope_evict(dst, ps, rows, cols, g):
                # dst = ps*A + lane_swap16(ps)*B  (rows 0:64 or 0:128)
                p16 = ropep.tile([128, SW], F16, name="p16")
                nc.scalar.copy(p16[:rows, :], ps[:rows, :])
                sw = ropep.tile([128, SW], F16, name="sw")
                nc.vector.stream_shuffle(sw[:rows, :], p16[:rows, :], SWAP16)
                u = ropep.tile([128, SW], F16, name="u")
                nc.vector.tensor_mul(
                    out=u[:rows, :], in0=p16[:rows, :], in1=ropea[:rows, cols]
                )
                v = ropep.tile([128, SW], F16, name="v")
                nc.vector.tensor_mul(
                    out=v[:rows, :], in0=sw[:rows, :], in1=ropeb[:rows, cols]
                )
                nc.gpsimd.tensor_add(out=dst, in0=u[:rows, :], in1=v[:rows, :])

            def qkv_group(g):
                cols = slice(g * SW, (g + 1) * SW)
                xs = xsp.tile([128, DO * SW], F16, name="xs")
                XQ = DO * SW // 2
                eng = nc.scalar if g == 1 else nc.sync
                for q in range(2):
                    eng.dma_start(
                        xs[:, XQ * q : XQ * (q + 1)],
                        x_ext[:, g * DO * SW + XQ * q : g * DO * SW + XQ * (q + 1)],
                    )
                for e in (2, 0, 1):  # k/v first: attention needs kT2 earliest
                    ps = projp.tile([128, SW], F32, name="pj")
                    for ko in range(DO):
                        nc.tensor.matmul(
                            ps,
                            wt_sb[:, ko * ECORE + e * 128 : ko * ECORE + (e + 1) * 128],
                            xs[:, ko * SW : (ko + 1) * SW],
                            start=(ko == 0),
                            stop=(ko == DO - 1),
                        )
                    if e < 2:
                        rope_evict(qq[e][:, cols], ps, 128, cols, g)
                    else:
                        rope_evict(kT2[0:HD, cols], ps, HD, cols, g)
                        nc.scalar.copy(kT2[HD:128, cols], kT2[0:HD, cols])
                        nc.vector.tensor_copy(vTh[:, cols], ps[HD:128, :])
                # ---- V transpose for this group's sk chunks ----
                for c in range(4 * g, 4 * g + 4):
                    pv = projp.tile([128, SW], F32, name="pj").bitcast(F16)[:, :HD]
                    nc.tensor.transpose(
                        pv[:], vTh[:, c * 128 : (c + 1) * 128], ident[:]
                    )
                    nc.vector.tensor_copy(v_sk[:, c, 0:HD], pv[:])

            def attn_pair(g, t):
                nblk = 4 * (g + 1)
                gwin = slice(g * SW, (g + 1) * SW)
                psy2 = [psyp.tile([HD + 1, SW], F32, name="y") for _ in range(2)]
                for b in range(nblk):
                    j = b - (nblk - 4)
                    blk = slice(b * 128, (b + 1) * 128)
                    # live subrange of the 512-wide sq window for this block
                    lsub = slice(128 * j, SW) if j > 0 else slice(0, SW)
                    pss = pssc.tile([128, 2, SW], F32, name="sc")
                    for half in range(2):
                        r = half * HD
                        nc.tensor.matmul(
                            pss[:, half, lsub],
                            kT2[r : r + HD, blk],
                            qq[t][r : r + HD, gwin][:, lsub],
                            start=True,
                            stop=True,
                            tile_position=(r, 0),
                        )
                    pt = ptp.tile([128, 2, SW], F16, name="pt")
                    if j >= 1:
                        nc.scalar.activation(
                            pt[:, :, lsub],
                            pss[:, :, lsub],
                            mybir.ActivationFunctionType.Exp,
                            scale=float(SCALE),
                        )
                    else:
                        nc.scalar.activation(
                            pt[:, :, :],
                            pss[:, :, :],
                            mybir.ActivationFunctionType.Exp,
                            scale=float(SCALE),
                        )
                    if j >= 0:
                        dsub = slice(128 * j, 128 * (j + 1))
                        nc.vector.tensor_mul(
                            out=pt[:, :, dsub], in0=pt[:, :, dsub], in1=tri[:, :, :]
                        )
                    for half in range(2):
                        nc.tensor.matmul(
                            psy2[half][:, lsub],
                            v_sk[:, b, :],
                            pt[:, half, lsub],
                            start=(b == 0),
                            stop=(b == nblk - 1),
                        )
                # ---- normalization: yT = psy / denom ----
                # evict psy to SBUF first so the PSUM banks free immediately
                # (the next pair's attn@V is gated on them); the reciprocal/
                # broadcast/divide then run off the critical path.
                for half in range(2):
                    psy = psy2[half]
                    base = half * HD
                    ev = work.tile([HD, SW], F32, name="ev")
                    nc.vector.tensor_copy(ev[:, :], psy[0:HD, :])
                    # denominator row to partition 0: reciprocal_approx_fast
                    # misbehaves on HW with a nonzero input partition base
                    dnr = work.tile([1, SW], F32, name="dnr")
                    nc.vector.tensor_copy(dnr[:], psy[HD : HD + 1, :])
                    rec = work.tile([1, SW], F32, name="rec")
                    nc.vector.reciprocal_approx_fast(rec[:], dnr[:])
                    bcs = work.tile([HD, SW], F32, name="bcs")
                    nc.gpsimd.partition_broadcast(bcs[:, :], rec[:, :], channels=HD)
                    nc.vector.tensor_mul(
                        out=yT[t][base : base + HD, gwin],
                        in0=ev[:, :],
                        in1=bcs[:, :],
                    )

            def wo_group(g):
                for sq in range(4 * g, 4 * g + 4):
                    ot = otp.tile([128, D], F16, name="ot")
                    for do in range(NG):
                        pso = projp.tile([128, SW], F32, name="pj")
                        for c in range(2):
                            nc.tensor.matmul(
                                pso[:],
                                yT[c][:, sq * 128 : (sq + 1) * 128],
                                wot[:, c, do * SW : (do + 1) * SW],
                                start=(c == 0),
                                stop=(c == 1),
                            )
                        if do == 0:
                            nc.scalar.copy(ot[:, do * SW : (do + 1) * SW], pso[:])
                        else:
                            nc.vector.tensor_copy(
                                ot[:, do * SW : (do + 1) * SW], pso[:]
                            )
                    nc.gpsimd.dma_start(
                        out_ext[sq * 128 : (sq + 1) * 128, :], ot[:]
                    )

            # interleave: PE filler work (qkv projections / output proj) is
            # emitted BETWEEN the two attention pairs of each group so the
            # scheduler can cover each pair's serial softmax tail.
            qkv_group(0)
            qkv_group(1)
            for g in range(NG):
                attn_pair(g, 0)
                if g < NG - 2:
                    qkv_group(g + 2)
                elif g >= 1:
                    wo_group(g - 1)
                attn_pair(g, 1)
                if g == 1:
                    wo_group(0)
            wo_group(NG - 1)

        warmup()
        if loop <= 1:
            body()
        else:
            with tc.For_i(0, loop, 1) as i:
                body(i)
    nc.finalize()
    return nc


class _CompiledSpmd:
    def __init__(self, nc: bass.Bass, n_cores: int = N_CORES):
        install_neuronx_cc_hook()
        self.nc = nc
        self.n_cores = n_cores
        partition_name = nc.partition_id_tensor.name if nc.partition_id_tensor else None

        in_names, out_names, out_avals, zero_shapes = [], [], [], []
        for alloc in nc.m.functions[0].allocations:
            if not isinstance(alloc, mybir.MemoryLocationSet):
                continue
            name = alloc.memorylocations[0].name
            if alloc.kind == "ExternalInput":
                if name != partition_name and name != (
                    nc.dbg_addr.name if nc.dbg_addr else None
                ):
                    in_names.append(name)
            elif alloc.kind == "ExternalOutput":
                out_names.append(name)
                shape = tuple(alloc.tensor_shape)
                dtype = mybir.dt.np(alloc.dtype)
                out_avals.append(jax.core.ShapedArray(shape, dtype))
                zero_shapes.append((shape, dtype))

        self.in_names, self.out_names = in_names, out_names
        self.out_avals, self.zero_shapes = out_avals, zero_shapes
        n_params, n_outs = len(in_names), len(out_names)

        full_in_names = list(in_names) + list(out_names)
        if nc.dbg_addr is not None:
            full_in_names.append(nc.dbg_addr.name)
        if partition_name is not None:
            full_in_names.append(partition_name)
        has_dbg = nc.dbg_addr is not None

        def _body(*args):
            operands = list(args)
            if has_dbg:
                operands.append(np.zeros((1, 2), np.uint32))
            if partition_name is not None:
                operands.append(partition_id_tensor())
            return tuple(
                _bass_exec_p.bind(
                    *operands,
                    out_avals=tuple(out_avals),
                    in_names=tuple(full_in_names),
                    out_names=tuple(out_names),
                    lowering_input_output_aliases=(),
                    sim_require_finite=True,
                    sim_require_nnan=True,
                    nc=nc,
                )
            )

        donate = tuple(range(n_params, n_params + n_outs))
        devices = jax.devices()[:n_cores]
        mesh = Mesh(np.asarray(devices), ("core",))
        self._fn = jax.jit(
            shard_map(
                _body,
                mesh=mesh,
                in_specs=(PartitionSpec("core"),) * (n_params + n_outs),
                out_specs=(PartitionSpec("core"),) * n_outs,
                check_rep=False,
            ),
            donate_argnums=donate,
            keep_unused=True,
        )

    def prep_inputs(self, in_maps):
        n = self.n_cores
        concat = [
            np.concatenate([np.asarray(in_maps[c][name]) for c in range(n)], axis=0)
            for name in self.in_names
        ]
        return [jax.device_put(a) for a in concat]

    def _zeros(self):
        return [
            np.zeros((self.n_cores * s[0], *s[1:]), d) for s, d in self.zero_shapes
        ]

    def run_prepped(self, dev_inputs):
        out = self._fn(*dev_inputs, *self._zeros())
        jax.block_until_ready(out)
        return out

    def run(self, in_maps):
        out_arrs = self.run_prepped(self.prep_inputs(in_maps))
        n = self.n_cores
        return [
            {
                name: np.asarray(out_arrs[i]).reshape(n, *self.out_avals[i].shape)[c]
                for i, name in enumerate(self.out_names)
            }
            for c in range(n)
        ]


# de-interleave pairs at 16-lane granularity: per 32-partition group,
# lanes 0:16 = real parts, 16:32 = imag parts (swap partner = lane XOR 16)
_PERM = np.concatenate(
    [
        np.arange(0, 32, 2),      # real of pairs 0..15
        np.arange(1, 32, 2),      # imag of pairs 0..15
        np.arange(32, 64, 2),     # real of pairs 16..31
        np.arange(33, 64, 2),     # imag of pairs 16..31
    ]
)


def make_in_maps(x, cos, sin, wqkv, wo):
    x = np.asarray(x, np.float32)
    cos = np.asarray(cos, np.float32)
    sin = np.asarray(sin, np.float32)
    wqkv = np.asarray(wqkv, np.float32)
    wo = np.asarray(wo, np.float32)

    # [128, NG, DO, SW] flattened: per-group loads are fully contiguous
    xt = np.ascontiguousarray(
        x[0].T.reshape(DO, 128, NG, SW).transpose(1, 2, 0, 3).reshape(128, -1)
    ).astype(np.float16)

    cosT, sinT = cos.T, sin.T  # [32, S]
    # 32-row periodic tables matching the 16-granular de-interleave:
    # rows 32a+b: b<16 -> freqs 16a+b (cos / -sin), b>=16 -> freqs 16a+b-16
    # (cos / +sin)
    a0 = np.concatenate([cosT[0:16], cosT[0:16]], axis=0)     # [32, S]
    a1 = np.concatenate([cosT[16:32], cosT[16:32]], axis=0)
    b0 = np.concatenate([-sinT[0:16], sinT[0:16]], axis=0)
    b1 = np.concatenate([-sinT[16:32], sinT[16:32]], axis=0)
    ropea = np.ascontiguousarray(np.tile(np.vstack([a0, a1]), (2, 1))).astype(
        np.float16
    )
    ropeb = np.ascontiguousarray(np.tile(np.vstack([b0, b1]), (2, 1))).astype(
        np.float16
    )

    pp, ff = np.arange(128)[:, None], np.arange(128)[None, :]
    trione = (ff >= pp).astype(np.float16)          # [128, 128] causal triangle
    trim = np.ascontiguousarray(
        np.stack([trione, trione], axis=1)
    ).astype(np.float16)                            # [128, 2, 128] both halves

    ident = np.eye(HD, dtype=np.float16)
    vones = np.ones((128, NSB, 1), np.float16)

    in_maps = []
    for i in range(N_CORES):
        wq = wqkv[i * EQ : (i + 1) * EQ].reshape(HQ, HD, D)[:, _PERM, :].reshape(
            EQ, D
        )
        wk = wqkv[NH * HD + i * HD : NH * HD + (i + 1) * HD][_PERM]
        wv = wqkv[NH * HD + NKV * HD + i * HD : NH * HD + NKV * HD + (i + 1) * HD]
        wcore = np.concatenate([wq, wk, wv], axis=0)  # [384, D]
        wt = np.ascontiguousarray(
            wcore.T.reshape(DO, 128, ECORE).transpose(1, 0, 2).reshape(128, -1)
        ).astype(np.float16)
        wos = wo[:, i * EQ : (i + 1) * EQ]  # [D, 256]
        wot = np.ascontiguousarray(
            wos.T.reshape(2, 128, D).transpose(1, 0, 2)
        ).astype(np.float16)
        in_maps.append(
            {
                "xt": xt,
                "wt": wt,
                "wot": wot,
                "ropea": ropea,
                "ropeb": ropeb,
                "trim": trim,
                "ident": ident,
                "vones": vones,
            }
        )
    return in_maps


_CACHE = {}


def get_compiled(loop: int = 1) -> _CompiledSpmd:
    if loop not in _CACHE:
        _CACHE[loop] = _CompiledSpmd(_build(loop))
    return _CACHE[loop]


def kernel(x, cos, sin, wqkv, wo):
    comp = get_compiled(1)
    in_maps = make_in_maps(x, cos, sin, wqkv, wo)
    results = comp.run(in_maps)
    acc = results[0]["out"].astype(np.float32)
    for c in range(1, N_CORES):
        acc += results[c]["out"].astype(np.float32)
    return acc.astype(np.float32).reshape(B, S, D)
